# revision 14
# baseline (speedup 1.0000x reference)
"""Trainium2 Bass kernel for the ConditionalPredicateHead GNN edge-MLP.

Per-edge computation (reference):
    out[e] = relu([emb[src[e]] | emb[tgt[e]] | u[batch[src[e]]]] @ W1 + b1) @ W2 + b2

Strategy (8 NeuronCores, edges data-parallel, 65536 edges/core):
  Decompose mm1 per NODE instead of per edge:
      Pa'[n] = emb[n] @ W1a + c[batch[n]]        (c = u @ W1c + b1)
      Pb [n] = emb[n] @ W1b
      h[e]   = relu(Pa'[src[e]] + Pb[tgt[e]])
      out[e] = h[e] @ W2 + b2
  Phase A (device): compute PaPb = [Pa' | Pb] fp16 for all 50176 (padded)
  nodes from a host-pre-transposed fp16 embedding table; the c-term is
  added during the PSUM->SBUF copy from a zero-interleaved gathered bias
  tile.  Rows are stored permuted (r(n) = s*1024 + p*8 + t for
  n = s*1024 + t*128 + p) so the partition-major SBUF store is one
  contiguous-per-partition DMA; host remaps edge indices to match.
  Phase A is sharded: each core computes 1/8 of the node table from its
  slice of embT, then a DRAM AllGather assembles the full table on every
  core (replica order == supertile order).
  Phase B (device): per 4096-edge chunk, 64 single-column indirect
  gathers (the toolchain disables vector_dynamic_offsets, so each gather
  carries one offset per partition; tgt rows reach the Pb half via
  element_offset), one fp16 add, a PE transpose per 128 edges, then mm2
  with W2 stationary (outT[9,512] per matmul) + b2 via scalar
  Identity-activation bias, fp16 out.

  No edge sorting or padding needed (the u/b1 bias rides on the nodes),
  so host prep is pure reshapes.
"""

import numpy as np
from contextlib import ExitStack

import concourse.bass as bass
import concourse.tile as tile
import concourse.mybir as mybir
from concourse.bass import IndirectOffsetOnAxis
from concourse.bass_utils import run_bass_kernel_spmd
from concourse.masks import make_identity

F32 = mybir.dt.float32
F16 = mybir.dt.float16
I32 = mybir.dt.int32

N_CORES = 8
E_FULL = 524288
N_NODES = 50000
HID = 128
GDIM = 8
NPRED = 9
NGRAPH = 64

E_CORE = E_FULL // N_CORES        # 65536
CHUNK = 4096                      # edges per phase-B chunk
KPP = CHUNK // 128                # 32 gathered rows per partition per chunk
NCHUNK = E_CORE // CHUNK          # 16
ASUP = 1024                       # nodes per phase-A supertile
NT = ASUP // 128                  # node tiles per supertile (8)
SUP_CORE = 7                      # phase-A supertiles computed per core
NASUP = SUP_CORE * N_CORES        # 56 supertiles total (sharded + AllGather)
NODES_PAD = NASUP * ASUP          # 57344
LOC_ROWS = SUP_CORE * 128         # papb_local rows (896)
ST = 512                          # edges per phase-B mm2 supertile
NST = CHUNK // ST                 # 8
LIMIT_WAITS = 1                   # walrus CoreV3 accepts at most one wait


def _split_excess_waits(nc, limit=1):
    """walrus CoreV3 codegen rejects instructions with more than `limit`
    semaphore waits; move extras onto injected same-engine nops placed
    right before the instruction (program order preserved per engine)."""
    n = 0
    for f in nc.m.functions:
        for b in f.blocks:
            out = []
            for inst in b.instructions:
                si = inst.sync_info
                waits = list(si.on_wait) if si is not None and si.on_wait else []
                if len(waits) > limit:
                    extra, keep = waits[:-limit], waits[-limit:]
                    for i in range(0, len(extra), limit):
                        nop = mybir.InstNoOp(
                            name=nc.get_next_instruction_name(),
                            ins=[], outs=[],
                            sync_info=mybir.SyncInfo(
                                on_wait=list(extra[i:i + limit]), on_update=[]),
                        )
                        nop.engine = inst.engine
                        nc.register_instruction(nop)
                        out.append(nop)
                        n += 1
                    si.on_wait = keep
                out.append(inst)
            b.instructions[:] = out
    return n


def build_nc():
    """Build the per-core SPMD Bass program (identical across cores)."""
    nc = bass.Bass(num_devices=N_CORES)
    embT = nc.dram_tensor("embT", [HID, SUP_CORE * ASUP], F16,
                          kind="ExternalInput")
    srcx = nc.dram_tensor("srcr", [128, NCHUNK * KPP], I32, kind="ExternalInput")
    tgtx = nc.dram_tensor("tgtr", [128, NCHUNK * KPP], I32, kind="ExternalInput")
    bidxx = nc.dram_tensor("bidx", [128, SUP_CORE * NT], I32,
                           kind="ExternalInput")
    u_x = nc.dram_tensor("u", [NGRAPH, GDIM], F32, kind="ExternalInput")
    w1abx = nc.dram_tensor("W1ab", [HID, 2 * HID], F16, kind="ExternalInput")
    w1cx = nc.dram_tensor("W1c", [GDIM, HID], F32, kind="ExternalInput")
    b1x = nc.dram_tensor("b1", [1, HID], F32, kind="ExternalInput")
    w2x = nc.dram_tensor("W2h", [HID, NPRED], F16, kind="ExternalInput")
    b2x = nc.dram_tensor("b2c", [NPRED, 1], F32, kind="ExternalInput")
    outx = nc.dram_tensor("out_shuf", [NCHUNK, NPRED, CHUNK], F16,
                          kind="ExternalOutput")

    with tile.TileContext(nc) as tc, ExitStack() as ctx:
        const = ctx.enter_context(tc.tile_pool(name="const", bufs=1))
        dramp = ctx.enter_context(
            tc.tile_pool(name="dram", bufs=1, space="DRAM"))
        # c / PaPb scratch as tracked DRAM tiles: the tile framework orders
        # the stores -> AllGather -> gathers without explicit barriers.
        # PaPb rows live permuted: row r = s*1024 + p*8 + t holds node
        # n = s*1024 + t*128 + p (gathers use axis=1 -> offset coef 256).
        c_dram = dramp.tile([NGRAPH, HID], F32, name="c_dram")
        papb_local = dramp.tile([LOC_ROWS, NT, 2 * HID], F16,
                                name="papb_local")
        papb = dramp.tile([NASUP * 128, NT, 2 * HID], F16, name="papb")

        ident16 = const.tile([128, 128], F16)
        make_identity(nc, ident16[:])
        ident32 = const.tile([128, 128], F32)
        make_identity(nc, ident32[:])

        w1ab = const.tile([HID, 2 * HID], F16, tag="w1ab")
        nc.sync.dma_start(out=w1ab[:], in_=w1abx[:, :])
        w2h = const.tile([HID, NPRED], F16, tag="w2h")
        nc.sync.dma_start(out=w2h[:], in_=w2x[:, :])
        b2c = const.tile([NPRED, 1], F32, tag="b2c")
        nc.sync.dma_start(out=b2c[:], in_=b2x[:, :])
        bidx = const.tile([128, SUP_CORE * NT], I32, tag="bidx")
        nc.sync.dma_start(out=bidx[:], in_=bidxx[:, :])
        src_i = const.tile([128, NCHUNK * KPP], I32, tag="src_i")
        nc.sync.dma_start(out=src_i[:], in_=srcx[:, :])
        tgt_i = const.tile([128, NCHUNK * KPP], I32, tag="tgt_i")
        nc.sync.dma_start(out=tgt_i[:], in_=tgtx[:, :])

        # c = u @ W1c + b1  (ones-augmented matmul), staged to DRAM for the
        # phase-A bias gather.
        with tc.tile_pool(name="setup", bufs=1) as sp, \
             tc.tile_pool(name="setup_ps", bufs=1, space="PSUM") as spp:
            rhs9 = sp.tile([GDIM + 1, HID], F32, tag="rhs9")
            nc.sync.dma_start(out=rhs9[0:GDIM, :], in_=w1cx[:, :])
            nc.sync.dma_start(out=rhs9[GDIM:GDIM + 1, :], in_=b1x[:, :])
            u_t = sp.tile([NGRAPH, GDIM], F32, tag="u_t")
            nc.sync.dma_start(out=u_t[:], in_=u_x[:, :])
            ps_ut = spp.tile([GDIM, NGRAPH], F32, space="PSUM", tag="ps_ut")
            nc.tensor.transpose(out=ps_ut[:], in_=u_t[:],
                                identity=ident32[0:NGRAPH, 0:NGRAPH])
            lhs9 = sp.tile([GDIM + 1, NGRAPH], F32, tag="lhs9")
            nc.vector.memset(lhs9[:], 1.0)
            nc.vector.tensor_copy(out=lhs9[0:GDIM, :], in_=ps_ut[:])
            ps_c = spp.tile([NGRAPH, HID], F32, space="PSUM", tag="ps_c")
            nc.tensor.matmul(out=ps_c[:], lhsT=lhs9[:], rhs=rhs9[:],
                             start=True, stop=True)
            c_sb = sp.tile([NGRAPH, HID], F32, tag="c_sb")
            nc.vector.tensor_copy(out=c_sb[:], in_=ps_c[:])
            nc.sync.dma_start(out=c_dram[:, :], in_=c_sb[:])

        # Cbc tiles: gathered c rows land in the Pa half, Pb half stays 0.
        cbc = [const.tile([128, NT, 2 * HID], F32, tag=f"cbc{i}",
                          name=f"cbc{i}") for i in range(2)]
        nc.vector.memset(cbc[0][:], 0.0)
        nc.vector.memset(cbc[1][:], 0.0)

        # ---- phase A: PaPb[n] = [emb @ W1a + c[batch] | emb @ W1b] ----
        with tc.tile_pool(name="pa_in", bufs=2) as pin, \
             tc.tile_pool(name="pa_out", bufs=2) as pout, \
             tc.tile_pool(name="pa_ps", bufs=2, space="PSUM") as pps:
            for s in range(SUP_CORE):
                et = pin.tile([128, ASUP], F16, tag="et")
                nc.sync.dma_start(out=et[:], in_=embT[:, s * ASUP:(s + 1) * ASUP])
                cb = cbc[s % 2]
                for t in range(NT):
                    nc.gpsimd.indirect_dma_start(
                        out=cb[:, t, 0:HID], out_offset=None,
                        in_=c_dram[:],
                        in_offset=IndirectOffsetOnAxis(
                            ap=bidx[:, s * NT + t:s * NT + t + 1], axis=0))
                ps_a = pps.tile([128, NT, 2 * HID], F32, space="PSUM", tag="ps_a")
                for t in range(NT):
                    nc.tensor.matmul(
                        out=ps_a[:, t, :],
                        lhsT=et[:, t * 128:(t + 1) * 128], rhs=w1ab[:],
                        start=True, stop=True)
                pp = pout.tile([128, NT, 2 * HID], F16, tag="pp")
                nc.vector.tensor_tensor(out=pp[:], in0=ps_a[:], in1=cb[:],
                                        op=mybir.AluOpType.add)
                nc.sync.dma_start(out=papb_local[s * 128:(s + 1) * 128],
                                  in_=pp[:])

        nc.gpsimd.collective_compute(
            "AllGather", mybir.AluOpType.bypass,
            replica_groups=[list(range(N_CORES))],
            ins=[papb_local.opt()], outs=[papb.opt()],
        )

        # ---- phase B: per-edge gather + relu + mm2 ----
        with tc.tile_pool(name="gath", bufs=2) as gp, \
             tc.tile_pool(name="work", bufs=2) as wp, \
             tc.tile_pool(name="outp", bufs=2) as op_, \
             tc.tile_pool(name="pb_ps", bufs=2, space="PSUM") as bps:
            for c in range(NCHUNK):
                ga = gp.tile([128, KPP, HID], F16, tag="ga")
                gb = gp.tile([128, KPP, HID], F16, tag="gb")
                for j in range(KPP):
                    nc.gpsimd.indirect_dma_start(
                        out=ga[:, j, :], out_offset=None,
                        in_=papb[:],
                        in_offset=IndirectOffsetOnAxis(
                            ap=src_i[:, c * KPP + j:c * KPP + j + 1], axis=1))
                    nc.gpsimd.indirect_dma_start(
                        out=gb[:, j, :], out_offset=None,
                        in_=papb[:],
                        in_offset=IndirectOffsetOnAxis(
                            ap=tgt_i[:, c * KPP + j:c * KPP + j + 1], axis=1),
                        element_offset=HID)
                s_t = gp.tile([128, KPP, HID], F16, tag="s_t")
                nc.vector.tensor_tensor(
                    out=s_t[:], in0=ga[:], in1=gb[:],
                    op=mybir.AluOpType.add)

                ot = op_.tile([NPRED, CHUNK], F16, tag="ot")
                for st in range(NST):
                    ps_t = bps.tile([128, ST], F16, space="PSUM", tag="ps_t")
                    for i in range(ST // 128):
                        j = st * (ST // 128) + i
                        nc.tensor.transpose(
                            out=ps_t[:, i * 128:(i + 1) * 128],
                            in_=s_t[:, j, :],
                            identity=ident16[:])
                    hT = wp.tile([128, ST], F16, tag="hT")
                    nc.vector.tensor_scalar_max(out=hT[:], in0=ps_t[:],
                                                scalar1=0.0)
                    ps_o = bps.tile([NPRED, ST], F32, space="PSUM", tag="ps_o")
                    nc.tensor.matmul(out=ps_o[:], lhsT=w2h[:], rhs=hT[:],
                                     start=True, stop=True)
                    nc.scalar.activation(
                        out=ot[:, st * ST:(st + 1) * ST], in_=ps_o[:],
                        func=mybir.ActivationFunctionType.Identity,
                        bias=b2c[:])
                nc.sync.dma_start(out=outx[c], in_=ot[:])

    _split_excess_waits(nc, limit=LIMIT_WAITS)
    return nc


# ---------------------------------------------------------------- host side

def _remap_rows(n):
    """Node id -> permuted PaPb row: n = s*1024 + t*128 + p -> s*1024 + p*8 + t."""
    s = n >> 10
    rem = n & 1023
    t = rem >> 7
    p = rem & 127
    return (s << 10) | (p << 3) | t


def _shuffle_idx(a):
    """[E_CORE] -> [128, NCHUNK*KPP] with out[p, c*KPP+j] = a[c*4096+j*128+p]."""
    return np.ascontiguousarray(
        a.reshape(NCHUNK, KPP, 128).transpose(2, 0, 1)).reshape(
            128, NCHUNK * KPP)


_NC_CACHE = {}


def _get_nc(gather_cast=True):
    # gather_cast kept for test.py compatibility; layout no longer needs it
    if "nc" not in _NC_CACHE:
        _NC_CACHE["nc"] = build_nc()
    return _NC_CACHE["nc"]


def make_in_maps(node_embeddings, edge_index, u, batch, W1, b1, W2, b2):
    emb = np.asarray(node_embeddings, dtype=np.float32)
    embT = np.zeros((HID, NODES_PAD), np.float16)
    embT[:, :N_NODES] = emb.astype(np.float16).T

    batch_np = np.asarray(batch).astype(np.int32)
    bpad = np.zeros(NODES_PAD, np.int32)
    bpad[:N_NODES] = batch_np
    # bidx[p, s*NT+t] = batch[s*1024 + t*128 + p]
    bidx = np.ascontiguousarray(
        bpad.reshape(NASUP, NT, 128).transpose(2, 0, 1)).reshape(
            128, NASUP * NT)

    W1_np = np.asarray(W1, dtype=np.float32)
    W1ab = np.ascontiguousarray(
        np.concatenate([W1_np[0:HID], W1_np[HID:2 * HID]], axis=1)
    ).astype(np.float16)
    W1c = np.ascontiguousarray(W1_np[2 * HID:])
    u_np = np.ascontiguousarray(np.asarray(u, dtype=np.float32))
    b1_np = np.asarray(b1, dtype=np.float32).reshape(1, HID)
    W2h = np.ascontiguousarray(np.asarray(W2, dtype=np.float32)
                               .astype(np.float16))
    b2c = np.ascontiguousarray(np.asarray(b2, dtype=np.float32)
                               .reshape(NPRED, 1))

    ei = np.asarray(edge_index)
    src_all = _remap_rows(ei[0].astype(np.int64)).astype(np.int32)
    tgt_all = _remap_rows(ei[1].astype(np.int64)).astype(np.int32)

    in_maps = []
    for c in range(N_CORES):
        sl = slice(c * E_CORE, (c + 1) * E_CORE)
        ns = slice(c * SUP_CORE * ASUP, (c + 1) * SUP_CORE * ASUP)
        in_maps.append({
            "embT": np.ascontiguousarray(embT[:, ns]),
            "srcr": _shuffle_idx(src_all[sl]),
            "tgtr": _shuffle_idx(tgt_all[sl]),
            "bidx": np.ascontiguousarray(
                bidx[:, c * SUP_CORE * NT:(c + 1) * SUP_CORE * NT]),
            "u": u_np, "W1ab": W1ab, "W1c": W1c, "b1": b1_np,
            "W2h": W2h, "b2c": b2c,
        })
    return in_maps, [None] * N_CORES


def assemble_output(results, metas):
    outs = []
    for c in range(N_CORES):
        o = np.asarray(results[c]["out_shuf"], dtype=np.float32)
        # [NCHUNK, NPRED, CHUNK] -> [NCHUNK*CHUNK, NPRED]
        outs.append(o.transpose(0, 2, 1).reshape(E_CORE, NPRED))
    return np.concatenate(outs, axis=0)


def kernel(node_embeddings, edge_index, u, batch, W1, b1, W2, b2):
    in_maps, metas = make_in_maps(node_embeddings, edge_index, u, batch,
                                  W1, b1, W2, b2)
    nc = _get_nc()
    res = run_bass_kernel_spmd(nc, in_maps, list(range(N_CORES)))
    return assemble_output(res.results, metas)
